# revision 1
# baseline (speedup 1.0000x reference)
"""Trainium2 Bass kernel for the blob-layer problem.

Computes out[b, c] = sum_hw x[b, hw] * curves[hw, c] / (H*W) where
curves[hw, c] = clip(factor_c * exp(-((xs-px_c)^2 + (ys-py_c)^2)/s2_c) * w_c).

Strategy (8 NeuronCores, SPMD):
- Shard the pixel (contraction) dim into 8 y-bands of 28 rows; each core
  computes a partial (B, C) output which the host sums.
- Per core, prune curve columns whose blob cannot reach its band
  (|py - band| > sqrt(T*s2)); contributions beyond that are < e^-T ~ 1e-11.
- grid is a rank-4 bilinear form:
    -grid = 2*px*xs + 2*py*ys - (px^2+py^2) - (xs^2+ys^2)
  so a K=4 fp32 matmul produces G = -grid for a 128-pixel tile against all
  kept columns. A DVE multiply by a replicated 1/s2 row gives M = -grid/s2
  (exact fp32; reduced-precision matmul is NOT usable here because 1/s2
  amplifies absolute error up to 1000x). ScalarE Exp produces e in bf16,
  and bf16 matmuls accumulate out[b, c] += x[hw, b] * e[hw, c] in PSUM.
- The clip never binds when max|factor*w| <= CAP (e <= 1), which holds for
  these inputs, so factor*w/npix is folded into a final per-column scale.
"""
import os
import sys

sys.path.insert(0, "/opt/trn_rl_repo")

import ml_dtypes
import numpy as np

import concourse.bass as bass
import concourse.bacc as bacc
import concourse.tile as tile
from concourse import mybir
from concourse.bass_utils import run_bass_kernel_spmd

H, W, B, C = 224, 224, 256, 1024
NDEV = 8
ROWS = H // NDEV          # 28 rows per band
HWD = ROWS * W            # 6272 pixels per band
NT = HWD // 128           # 49 pixel tiles per band
EPS = 0.001
CAP = 2000.0
NPIX = float(H * W)
T_PRUNE = 25.0            # exp(-25) ~ 1.4e-11: dropped-column contribution bound

last_results = None       # BassKernelResults of the most recent run (for profiling)


def _build_program(nc_cols, reps=1, hw_loop=False, skip_g=False, skip_main=False,
                   skip_act=False, skew=2):
    """Emit the SPMD Bass program for NC kept/padded columns per core.

    Sync-wait discipline: a fused fp32 LDWEIGHTS+MATMUL accepts only ONE
    semaphore wait, so every fp32 matmul may depend on at most one new tick.
    The A matrix is pre-scaled by 1/s2 so the K=4 fp32 matmul yields
    M = -grid/s2 directly in PSUM and ScalarE's Exp reads PSUM — no DVE
    stage. The G matmul's only dep is then a single PE sem value (PSUM slot
    release merged with the weight-register WAR); its ACT release is already
    observed via the preceding main matmul's e-wait. bf16 main matmuls get a
    split LDWEIGHTS, so their x-DMA wait and e-ACT wait land on separate
    instructions. Bm/Am share one DMA so the first G matmul sees one queue
    sem.
    """
    NC = nc_cols
    nc = bacc.Bacc()
    f32 = mybir.dt.float32
    f16 = mybir.dt.float16
    bf16 = mybir.dt.bfloat16

    d_xT = nc.declare_dram_parameter("xT", [NT, 128, B], bf16, isOutput=False)
    d_Wp = nc.declare_dram_parameter("Wp", [12, HWD], f16, isOutput=False)
    d_Mv = nc.declare_dram_parameter("Mv", [12, NC], f16, isOutput=False)
    d_Fw = nc.declare_dram_parameter("Fw", [128, NC], f32, isOutput=False)
    d_out = nc.declare_dram_parameter("out", [2, 128, NC], f32, isOutput=True)

    c_chunks = [(c0, min(512, NC - c0)) for c0 in range(0, NC, 512)]

    with tile.TileContext(nc) as tc:
        with (
            tc.tile_pool(name="const", bufs=1) as cpool,
            tc.tile_pool(name="ep", bufs=4) as ep,
            tc.tile_pool(name="op", bufs=1) as op,
            tc.tile_pool(name="psG", bufs=4, space="PSUM") as psG,
            tc.tile_pool(name="psO", bufs=1, space="PSUM") as psO,
        ):
            Wp = cpool.tile([12, HWD], f16, tag="Wp")
            Mv = cpool.tile([12, NC], f16, tag="Mv")
            Fw = cpool.tile([128, NC], f32, tag="Fw")
            nc.gpsimd.dma_start(Wp[:], d_Wp[:])
            nc.gpsimd.dma_start(Mv[:], d_Mv[:])
            nc.gpsimd.dma_start(Fw[:], d_Fw[:])

            # whole x band stays SBUF-resident (25KB/partition): a few large
            # DMAs write disjoint ranges of one tile, so no slot-recycle or
            # queue-ring waits exist and each main LDWEIGHTS waits on at most
            # one DMA queue sem.
            xfull = cpool.tile([128, NT * B], bf16, tag="xfull")
            grp = (NT + 6) // 7
            for t0 in range(0, NT, grp):
                t1 = min(t0 + grp, NT)
                nc.sync.dma_start(
                    xfull[:, t0 * B : t1 * B].rearrange(
                        "p (t b) -> p t b", t=t1 - t0
                    ),
                    d_xT[t0:t1].rearrange("t p b -> p t b"),
                )

            Op0 = psO.tile([128, NC], f32, tag="op0")
            Op1 = psO.tile([128, NC], f32, tag="op1")

            def emit_main(j, e):
                first, last = j == 0, j == NT - 1
                for bb, Opx in ((0, Op0), (1, Op1)):
                    for c0, cw in c_chunks:
                        nc.tensor.matmul(
                            Opx[:, c0 : c0 + cw],
                            xfull[:, j * B + bb * 128 : j * B + (bb + 1) * 128],
                            e[:, c0 : c0 + cw],
                            start=first,
                            stop=last,
                            skip_group_check=True,
                        )

            def one_pass():
                pending = []
                for t in range(NT):
                    Gp = psG.tile([128, NC], f32, tag="Gp")
                    if not skip_g:
                        # M = -grid/s2 via 3-term fp16 hi/lo split (hi*hi +
                        # hi*lo + lo*hi ~ 2^-22 precision; fp16 runs at full
                        # PE rate while fp32 measured ~8x slower; subnormals
                        # honored — probed). The three K=4 terms are STACKED
                        # along the contraction dim (K=12, rows [Bh;Bh;Bl] x
                        # [Ah;Al;Ah]) so one matmul computes the whole sum at
                        # the cost of a K=4 one.
                        for c0, cw in c_chunks:
                            nc.tensor.matmul(
                                Gp[:, c0 : c0 + cw],
                                Wp[:, t * 128 : (t + 1) * 128],
                                Mv[:, c0 : c0 + cw],
                                start=True,
                                stop=True,
                                skip_group_check=True,
                            )
                    e = ep.tile([128, NC], bf16, tag="e")
                    if skip_act:
                        if t < skew + 1:
                            nc.scalar.activation(
                                e[:], Gp[:], mybir.ActivationFunctionType.Exp
                            )
                    else:
                        nc.scalar.activation(
                            e[:], Gp[:], mybir.ActivationFunctionType.Exp
                        )

                    pending.append((t, e))
                    if len(pending) > skew and not skip_main:
                        emit_main(*pending.pop(0))
                if not skip_main:
                    while pending:
                        emit_main(*pending.pop(0))

            # reps>1 repeats the identical computation (timing harness only;
            # each rep's start=True resets the accumulators, so the final
            # output is unchanged).
            if hw_loop and reps > 1:
                with tc.For_i(0, reps, 1):
                    one_pass()
            else:
                for _ in range(reps):
                    one_pass()

            out_sb = op.tile([128, 2 * NC], f32, tag="out")
            nc.vector.tensor_mul(out_sb[:, 0:NC], Op0[:], Fw[:])
            nc.vector.tensor_mul(out_sb[:, NC : 2 * NC], Op1[:], Fw[:])
            nc.sync.dma_start(d_out[0], out_sb[:, 0:NC])
            nc.sync.dma_start(d_out[1], out_sb[:, NC : 2 * NC])

    nc.compile()
    _dedup_ldweights(nc)
    return nc


def _dedup_ldweights(nc):
    """Drop LDWEIGHTS that reload the exact weights already resident in the
    PE array (same AP, no intervening load, no sems). ~107ns each on HW; the
    cost model prices them at 0 so Tile never minimizes them."""
    for f in nc.m.functions:
        for blk in f.blocks:
            keep = []
            prev_ap = None
            for inst in blk.instructions:
                tn = type(inst).__name__
                if tn == "InstLdweights":
                    si = inst.sync_info
                    w = (si.on_wait if si else []) or []
                    u = (si.on_update if si else []) or []
                    ap = repr(inst.ins[0])
                    if ap == prev_ap and not w and not u:
                        continue
                    prev_ap = ap
                keep.append(inst)
            if len(keep) != len(blk.instructions):
                del blk.instructions[:]
                blk.instructions.extend(keep)


def _prepare(x, positions, sigmas, curve_weights, xs, ys):
    x = np.asarray(x, dtype=np.float32)
    px = np.asarray(positions, dtype=np.float64)[0, 0, :, 1]
    py = np.asarray(positions, dtype=np.float64)[0, 0, :, 0]
    sg = np.asarray(sigmas, dtype=np.float64)[0, 0]
    w = np.asarray(curve_weights, dtype=np.float64)[0, 0]
    xs = np.asarray(xs, dtype=np.float64)
    ys = np.asarray(ys, dtype=np.float64)

    s2 = 2.0 * sg * sg + EPS
    factor = 1.0 / (2.0 * np.pi * sg * sg + EPS)
    fw = factor * w
    # clip(curves) is identity when max|factor*w| <= CAP since exp(...) <= 1
    assert np.abs(fw).max() <= CAP, "clip binds; folded-scale scheme invalid"

    # Per band keep columns whose blob reaches it (margin^2/s2 <= T), capped
    # at 512 (one PSUM bank) by dropping the weakest-coupled columns; the
    # implied threshold of dropped columns stays >= ~10 (contribution < 1e-4
    # relative).
    keep_idx = []
    for d in range(NDEV):
        h0 = d * ROWS
        y0, y1 = ys[h0, 0], ys[h0 + ROWS - 1, 0]
        margin = np.maximum(np.maximum(y0 - py, py - y1), 0.0)
        score = margin * margin / s2
        idx = np.where(score <= T_PRUNE)[0]
        if len(idx) > 512:
            idx = idx[np.argsort(score[idx], kind="stable")[:512]]
            idx.sort()
        keep_idx.append(idx)
    NC = max(128, -(-max(len(i) for i in keep_idx) // 128) * 128)
    assert NC <= 512

    in_maps = []
    for d in range(NDEV):
        h0 = d * ROWS
        rows = slice(h0, h0 + ROWS)
        xs_b = xs[rows].ravel()
        ys_b = ys[rows].ravel()
        Bm = np.stack(
            [xs_b, ys_b, np.ones(HWD), xs_b * xs_b + ys_b * ys_b]
        ).astype(np.float32)

        idx = keep_idx[d]
        nk = len(idx)
        # A columns pre-scaled by 1/s2 so the matmul yields M = -grid/s2
        Am = np.zeros((4, NC), np.float32)
        Am[0, :nk] = 2.0 * px[idx] / s2[idx]
        Am[1, :nk] = 2.0 * py[idx] / s2[idx]
        Am[2, :nk] = -(px[idx] ** 2 + py[idx] ** 2) / s2[idx]
        Am[3, :nk] = -1.0 / s2[idx]
        Am[3, nk:] = -1.0
        Bm64 = Bm.astype(np.float64)
        Am64 = Am.astype(np.float64)
        Bh = Bm64.astype(np.float16)
        Bl = (Bm64 - Bh.astype(np.float64)).astype(np.float16)
        Ah = Am64.astype(np.float16)
        Al = (Am64 - Ah.astype(np.float64)).astype(np.float16)
        # K=12 stacked hi/lo split: [Bh;Bh;Bl]^T @ [Ah;Al;Ah]
        Wp = np.concatenate([Bh, Bh, Bl], axis=0)
        Mv = np.concatenate([Ah, Al, Ah], axis=0)
        F = np.zeros(NC, np.float64)
        F[:nk] = fw[idx] / NPIX

        xT = np.ascontiguousarray(
            x[:, rows, :].reshape(B, HWD).T
        ).reshape(NT, 128, B).astype(ml_dtypes.bfloat16)

        in_maps.append(
            {
                "xT": xT,
                "Wp": Wp,
                "Mv": Mv,
                "Fw": np.ascontiguousarray(
                    np.broadcast_to(F.astype(np.float32), (128, NC))
                ),
            }
        )
    return NC, in_maps, keep_idx


def _gather(results, keep_idx, NC):
    out = np.zeros((B, C), np.float32)
    for d in range(NDEV):
        nk = len(keep_idx[d])
        dev = np.asarray(results[d]["out"], np.float32).reshape(B, NC)
        out[:, keep_idx[d]] += dev[:, :nk]
    return out


def kernel(x, positions, sigmas, curve_weights, xs, ys):
    global last_results
    NC, in_maps, keep_idx = _prepare(x, positions, sigmas, curve_weights, xs, ys)
    nc = _build_program(NC)
    trace = bool(os.environ.get("BLOB_TRACE"))
    last_results = run_bass_kernel_spmd(
        nc, in_maps, list(range(NDEV)), trace=trace
    )
    return _gather(last_results.results, keep_idx, NC)



# revision 4
# speedup vs baseline: 2.5741x; 2.5741x over previous
"""Trainium2 Bass kernel for the blob-layer problem.

Computes out[b, c] = sum_hw x[b, hw] * curves[hw, c] / (H*W) where
curves[hw, c] = clip(factor_c * exp(-((xs-px_c)^2 + (ys-py_c)^2)/s2_c) * w_c).

Strategy (8 NeuronCores, SPMD, one shared program):
- Shard the pixel (contraction) dim into 8 y-bands of 28 rows; each core
  computes a partial (B, NC) output in slot space which the host scatters.
- Band pixels are laid out x-major (column-major image order), so each
  128-pixel tile spans only ~4.6 of the 224 x-columns. Curves are sorted
  by px and assigned to NC slots following the global px-quantile, so the
  set of curves within reach of a tile is a short contiguous slot window
  that is aligned across cores (the program, and hence the static window
  bounds, is shared by all 8 cores; per-tile windows are the union of the
  per-core windows).
- Pruning is amplitude-aware: curve c is kept for margin m when
  m^2/s2_c <= T + ln(|fw_c|/max|fw|) (T_KEEP for the y-band criterion,
  T_WIN for the per-tile x window); dropped contributions are bounded by
  e^-T * max|fw| / npix per pixel.
- grid is a rank-4 bilinear form: a K=12 stacked fp16 hi/lo matmul
  (rows [Bh;Bh;Bl] x [Ah;Al;Ah], ~2^-22 precision at full PE rate)
  produces M = -grid/s2 + ln(|fw_c|/npix) for a 128-pixel tile against
  its slot window directly in PSUM. ScalarE Exp gives e = curves_c/npix
  (up to sign) in bf16, and bf16 matmuls accumulate x^T e into two
  128-batch PSUM banks. The per-curve sign is applied host-side during
  the gather, so no on-device rescale pass is needed.
- PSUM output banks are zero-filled once up front and every main matmul
  accumulates (start=False): a slot's first touching tile varies per
  slot, so start-flag zeroing can't be used.
- The clip never binds when max|factor*w| <= CAP (e <= 1), which holds
  for these inputs (asserted host-side).
"""
import os
import sys

sys.path.insert(0, "/opt/trn_rl_repo")

import ml_dtypes
import numpy as np

import concourse.bass as bass
import concourse.bacc as bacc
import concourse.tile as tile
from concourse import mybir
from concourse.bass_utils import run_bass_kernel_spmd

H, W, B, C = 224, 224, 256, 1024
NDEV = 8
ROWS = H // NDEV          # 28 rows per band
HWD = ROWS * W            # 6272 pixels per band
NT = HWD // 128           # 49 pixel tiles per band
EPS = 0.001
CAP = 2000.0
NPIX = float(H * W)
NC = 480                  # slot count (fits one 2KB fp32 PSUM bank)
T_KEEP = 7.0              # y-band keep threshold (amplitude-adjusted)
T_WIN = 5.0               # per-tile x-window threshold (amplitude-adjusted)
T_FLOOR = 1.0

last_results = None       # BassKernelResults of the most recent run (for profiling)


def _build_program(windows, skew=3):
    """Emit the SPMD Bass program. `windows` is a list of NT (lo, hi) slot
    ranges (shared across cores)."""
    nc = bacc.Bacc()
    f32 = mybir.dt.float32
    f16 = mybir.dt.float16
    bf16 = mybir.dt.bfloat16
    wmax = max((hi - lo for lo, hi in windows), default=1)

    d_xT = nc.declare_dram_parameter("xT", [NT, 128, B], bf16, isOutput=False)
    d_Wp = nc.declare_dram_parameter("Wp", [12, HWD], f16, isOutput=False)
    d_Mv = nc.declare_dram_parameter("Mv", [12, NC], f16, isOutput=False)
    d_out = nc.declare_dram_parameter("out", [2, 128, NC], f32, isOutput=True)

    with tile.TileContext(nc) as tc:
        with (
            tc.tile_pool(name="const", bufs=1) as cpool,
            tc.tile_pool(name="ep", bufs=6) as ep,
            tc.tile_pool(name="op", bufs=1) as op,
            tc.tile_pool(name="psG", bufs=6, space="PSUM") as psG,
            tc.tile_pool(name="psO", bufs=1, space="PSUM") as psO,
        ):
            Wp = cpool.tile([12, HWD], f16, tag="Wp")
            Mv = cpool.tile([12, NC], f16, tag="Mv")
            nc.gpsimd.dma_start(Wp[:], d_Wp[:])
            nc.gpsimd.dma_start(Mv[:], d_Mv[:])

            # whole x band stays SBUF-resident (25KB/partition); chunks are
            # spread round-robin over four engine DMA queues so the load
            # runs at aggregate bandwidth and the first tiles land early.
            xfull = cpool.tile([128, NT * B], bf16, tag="xfull")
            queues = [nc.sync, nc.scalar, nc.gpsimd]
            grp = 5
            for k, t0 in enumerate(range(0, NT, grp)):
                t1 = min(t0 + grp, NT)
                queues[k % 3].dma_start(
                    xfull[:, t0 * B : t1 * B].rearrange(
                        "p (t b) -> p t b", t=t1 - t0
                    ),
                    d_xT[t0:t1].rearrange("t p b -> p t b"),
                )

            Op0 = psO.tile([128, NC], f32, tag="op0")
            Op1 = psO.tile([128, NC], f32, tag="op1")
            nc.vector.memset(Op0[:], 0.0)
            nc.vector.memset(Op1[:], 0.0)

            def emit_main(t, lo, hi, e):
                last = t == NT - 1
                w = hi - lo
                for bb, Opx in ((0, Op0), (1, Op1)):
                    nc.tensor.matmul(
                        Opx[:, lo:hi],
                        xfull[:, t * B + bb * 128 : t * B + (bb + 1) * 128],
                        e[:, 0:w],
                        start=False,
                        stop=last,
                        skip_group_check=True,
                    )

            pending = []
            for t in range(NT):
                lo, hi = windows[t]
                w = hi - lo
                if w > 0:
                    Gp = psG.tile([128, wmax], f32, tag="Gp")
                    nc.tensor.matmul(
                        Gp[:, 0:w],
                        Wp[:, t * 128 : (t + 1) * 128],
                        Mv[:, lo:hi],
                        start=True,
                        stop=True,
                        skip_group_check=True,
                    )
                    e = ep.tile([128, wmax], bf16, tag="e")
                    nc.scalar.activation(
                        e[:, 0:w], Gp[:, 0:w], mybir.ActivationFunctionType.Exp
                    )
                    pending.append((t, lo, hi, e))
                if len(pending) > skew:
                    emit_main(*pending.pop(0))
            while pending:
                emit_main(*pending.pop(0))

            out_sb = op.tile([128, 2 * NC], f32, tag="out")
            nc.scalar.copy(out_sb[:, 0:NC], Op0[:])
            nc.vector.tensor_copy(out_sb[:, NC : 2 * NC], Op1[:])
            nc.sync.dma_start(d_out[0], out_sb[:, 0:NC])
            nc.scalar.dma_start(d_out[1], out_sb[:, NC : 2 * NC])

    nc.compile()
    return nc


def _prepare(x, positions, sigmas, curve_weights, xs, ys):
    x = np.asarray(x, dtype=np.float32)
    px = np.asarray(positions, dtype=np.float64)[0, 0, :, 1]
    py = np.asarray(positions, dtype=np.float64)[0, 0, :, 0]
    sg = np.asarray(sigmas, dtype=np.float64)[0, 0]
    w = np.asarray(curve_weights, dtype=np.float64)[0, 0]
    xs = np.asarray(xs, dtype=np.float64)
    ys = np.asarray(ys, dtype=np.float64)

    s2 = 2.0 * sg * sg + EPS
    factor = 1.0 / (2.0 * np.pi * sg * sg + EPS)
    fw = factor * w
    # clip(curves) is identity when max|factor*w| <= CAP since exp(...) <= 1
    assert np.abs(fw).max() <= CAP, "clip binds; folded-scale scheme invalid"

    absfw = np.maximum(np.abs(fw), 1e-12)
    lnr = np.log(absfw / absfw.max())
    Tk = np.maximum(T_KEEP + lnr, T_FLOOR)
    Tw = np.maximum(T_WIN + lnr, T_FLOOR)

    gorder = np.argsort(px, kind="stable")
    grank = np.empty(C, dtype=np.int64)
    grank[gorder] = np.arange(C)

    los = np.full((NDEV, NT), np.iinfo(np.int64).max, dtype=np.int64)
    his = np.zeros((NDEV, NT), dtype=np.int64)
    band = []
    for d in range(NDEV):
        h0 = d * ROWS
        y0, y1 = ys[h0, 0], ys[h0 + ROWS - 1, 0]
        ymarg = np.maximum(np.maximum(y0 - py, py - y1), 0.0)
        kept = np.where(ymarg * ymarg / s2 <= Tk)[0]
        order = kept[np.argsort(px[kept], kind="stable")]
        nk = len(order)
        assert nk <= NC, f"band {d} keeps {nk} > NC={NC} columns"
        # monotone slot assignment following the global px-quantile so the
        # per-tile windows line up across bands
        ideal = (grank[order] * NC) // C
        slot = np.zeros(nk, dtype=np.int64)
        s = -1
        for i in range(nk):
            s = max(s + 1, int(ideal[i]))
            slot[i] = s
        if nk and slot[-1] > NC - 1:
            slot[-1] = NC - 1
            for i in range(nk - 2, -1, -1):
                slot[i] = min(slot[i], slot[i + 1] - 1)
        pxs = px[order]
        ym = ymarg[order]
        for t in range(NT):
            xi0, xi1 = (t * 128) // ROWS, (t * 128 + 127) // ROWS
            xx0, xx1 = xs[0, xi0], xs[0, min(xi1, W - 1)]
            xmarg = np.maximum(np.maximum(xx0 - pxs, pxs - xx1), 0.0)
            act = np.where((ym * ym + xmarg * xmarg) / s2[order] <= Tw[order])[0]
            if len(act):
                los[d, t] = slot[act[0]]
                his[d, t] = slot[act[-1]] + 1
        band.append((order, slot))

    lo_u = los.min(axis=0)
    hi_u = his.max(axis=0)
    # enforce monotone windows (they already are, up to ties) so every slot
    # in [lo_0, hi_last) is covered by a contiguous run of tiles
    for t in range(1, NT):
        hi_u[t] = max(hi_u[t], hi_u[t - 1]) if hi_u[t] else hi_u[t - 1]
        lo_u[t] = max(min(lo_u[t], hi_u[t]), lo_u[t - 1])
    windows = [
        (int(min(lo_u[t], hi_u[t])), int(hi_u[t])) for t in range(NT)
    ]

    in_maps = []
    gathers = []
    for d in range(NDEV):
        h0 = d * ROWS
        rows = slice(h0, h0 + ROWS)
        # x-major pixel order: p = xi*ROWS + yi
        xs_b = xs[rows].T.ravel()
        ys_b = ys[rows].T.ravel()
        Bm = np.stack(
            [xs_b, ys_b, np.ones(HWD), xs_b * xs_b + ys_b * ys_b]
        )

        order, slot = band[d]
        lnf = np.log(np.abs(fw[order]) + 1e-300) - np.log(NPIX)
        Am = np.zeros((4, NC))
        Am[2, :] = -60.0
        Am[3, :] = -1.0
        Am[0, slot] = 2.0 * px[order] / s2[order]
        Am[1, slot] = 2.0 * py[order] / s2[order]
        Am[2, slot] = -(px[order] ** 2 + py[order] ** 2) / s2[order] + lnf
        Am[3, slot] = -1.0 / s2[order]
        Bh = Bm.astype(np.float16)
        Bl = (Bm - Bh.astype(np.float64)).astype(np.float16)
        Ah = Am.astype(np.float16)
        Al = (Am - Ah.astype(np.float64)).astype(np.float16)
        # K=12 stacked hi/lo split: [Bh;Bh;Bl]^T @ [Ah;Al;Ah]
        Wp = np.concatenate([Bh, Bh, Bl], axis=0)
        Mv = np.concatenate([Ah, Al, Ah], axis=0)

        xT = np.ascontiguousarray(
            x[:, rows, :].transpose(0, 2, 1).reshape(B, HWD).T
        ).reshape(NT, 128, B).astype(ml_dtypes.bfloat16)

        in_maps.append({"xT": xT, "Wp": Wp, "Mv": Mv})
        gathers.append((order, slot, np.sign(fw[order]).astype(np.float32)))
    return windows, in_maps, gathers


def _gather(results, gathers):
    out = np.zeros((B, C), np.float32)
    for d in range(NDEV):
        order, slot, sgn = gathers[d]
        dev = np.asarray(results[d]["out"], np.float32).reshape(B, NC)
        out[:, order] += dev[:, slot] * sgn
    return out


def kernel(x, positions, sigmas, curve_weights, xs, ys):
    global last_results
    windows, in_maps, gathers = _prepare(
        x, positions, sigmas, curve_weights, xs, ys
    )
    nc = _build_program(windows)
    trace = bool(os.environ.get("BLOB_TRACE"))
    last_results = run_bass_kernel_spmd(
        nc, in_maps, list(range(NDEV)), trace=trace
    )
    return _gather(last_results.results, gathers)


# revision 5
# speedup vs baseline: 2.8994x; 1.1264x over previous
"""Trainium2 Bass kernel for the blob-layer problem.

Computes out[b, c] = sum_hw x[b, hw] * curves[hw, c] / (H*W) where
curves[hw, c] = clip(factor_c * exp(-((xs-px_c)^2 + (ys-py_c)^2)/s2_c) * w_c).

Strategy (8 NeuronCores, SPMD, one shared program):
- Shard the pixel (contraction) dim into 8 y-bands of 28 rows; each core
  computes a partial (B, NC) output in slot space which the host scatters.
- Band pixels are laid out x-major (column-major image order), so each
  128-pixel tile spans only ~4.6 of the 224 x-columns. Curves are sorted
  by px and assigned to NC slots following the global px-quantile, so the
  set of curves within reach of a tile is a short contiguous slot window
  that is aligned across cores (the program, and hence the static window
  bounds, is shared by all 8 cores; per-tile windows are the union of the
  per-core windows).
- Pruning is amplitude-aware: curve c is kept for margin m when
  m^2/s2_c <= T + ln(|fw_c|/max|fw|) (T_KEEP for the y-band criterion,
  T_WIN for the per-tile x window); dropped contributions are bounded by
  e^-T * max|fw| / npix per pixel.
- grid is a rank-4 bilinear form: a K=12 stacked fp16 hi/lo matmul
  (rows [Bh;Bh;Bl] x [Ah;Al;Ah], ~2^-22 precision at full PE rate)
  produces M = -grid/s2 + ln(|fw_c|/npix) for a 128-pixel tile against
  its slot window directly in PSUM. ScalarE Exp gives e = curves_c/npix
  (up to sign) in bf16, and bf16 matmuls accumulate x^T e into two
  128-batch PSUM banks. The per-curve sign is applied host-side during
  the gather, so no on-device rescale pass is needed.
- PSUM output banks are zero-filled once up front and every main matmul
  accumulates (start=False): a slot's first touching tile varies per
  slot, so start-flag zeroing can't be used.
- The clip never binds when max|factor*w| <= CAP (e <= 1), which holds
  for these inputs (asserted host-side).
"""
import os
import sys

sys.path.insert(0, "/opt/trn_rl_repo")

import ml_dtypes
import numpy as np

import concourse.bass as bass
import concourse.bacc as bacc
import concourse.tile as tile
from concourse import mybir
from concourse.bass_utils import run_bass_kernel_spmd

H, W, B, C = 224, 224, 256, 1024
NDEV = 8
ROWS = H // NDEV          # 28 rows per band
HWD = ROWS * W            # 6272 pixels per band
NT = HWD // 128           # 49 pixel tiles per band
EPS = 0.001
CAP = 2000.0
NPIX = float(H * W)
NC = 480                  # slot count (fits one 2KB fp32 PSUM bank)
T_KEEP = 7.0              # y-band keep threshold (amplitude-adjusted)
T_WIN = 5.0               # per-tile x-window threshold (amplitude-adjusted)
T_FLOOR = 1.0

last_results = None       # BassKernelResults of the most recent run (for profiling)


def _build_program(windows, skew=2, grp_t=3):
    """Emit the SPMD Bass program. `windows` is a list of NT (lo, hi) slot
    ranges (shared across cores). Tiles are processed in groups of grp_t:
    each group's G matmuls pack their outputs contiguously into one PSUM
    bank so a single ScalarE Exp covers the whole group (the ~250ns fixed
    cost per ACTIVATE dominates at these window widths)."""
    nc = bacc.Bacc()
    f32 = mybir.dt.float32
    f16 = mybir.dt.float16
    bf16 = mybir.dt.bfloat16

    d_xT = nc.declare_dram_parameter("xT", [NT, 128, B], bf16, isOutput=False)
    d_Wp = nc.declare_dram_parameter("Wp", [12, HWD], f16, isOutput=False)
    d_Mv = nc.declare_dram_parameter("Mv", [12, NC], f16, isOutput=False)
    d_out = nc.declare_dram_parameter("out", [2, 128, NC], f32, isOutput=True)

    # groups of tiles whose packed G widths fit one 512-fp32 PSUM bank
    groups = []
    cur, acc = [], 0
    for t in range(NT):
        lo, hi = windows[t]
        w = hi - lo
        if w == 0:
            continue
        if len(cur) == grp_t or acc + w > 512:
            groups.append(cur)
            cur, acc = [], 0
        cur.append((t, lo, hi, acc))
        acc += w
    if cur:
        groups.append(cur)

    with tile.TileContext(nc) as tc:
        with (
            tc.tile_pool(name="const", bufs=1) as cpool,
            tc.tile_pool(name="ep", bufs=4) as ep,
            tc.tile_pool(name="op", bufs=1) as op,
            tc.tile_pool(name="psG", bufs=4, space="PSUM") as psG,
            tc.tile_pool(name="psO", bufs=1, space="PSUM") as psO,
        ):
            Wp = cpool.tile([12, HWD], f16, tag="Wp")
            Mv = cpool.tile([12, NC], f16, tag="Mv")
            # Mv (tiny) and the first Wp half gate the first G matmul: they
            # ride the sync queue ahead of everything else. x chunks start
            # small so early tiles land quickly, then grow.
            nc.sync.dma_start(Mv[:], d_Mv[:])
            nc.sync.dma_start(Wp[:, 0 : 25 * 128], d_Wp[:, 0 : 25 * 128])
            nc.scalar.dma_start(Wp[:, 25 * 128 :], d_Wp[:, 25 * 128 :])

            # whole x band stays SBUF-resident (25KB/partition); chunks are
            # spread over three engine DMA queues so the load runs at
            # aggregate bandwidth and the first tiles land early.
            xfull = cpool.tile([128, NT * B], bf16, tag="xfull")
            chunk_plan = [
                (nc.gpsimd, 0, 3), (nc.scalar, 3, 6), (nc.gpsimd, 6, 9),
                (nc.sync, 9, 15), (nc.scalar, 15, 21), (nc.gpsimd, 21, 27),
                (nc.sync, 27, 35), (nc.scalar, 35, 42), (nc.gpsimd, 42, 49),
            ]
            for q, t0, t1 in chunk_plan:
                q.dma_start(
                    xfull[:, t0 * B : t1 * B].rearrange(
                        "p (t b) -> p t b", t=t1 - t0
                    ),
                    d_xT[t0:t1].rearrange("t p b -> p t b"),
                )

            Op0 = psO.tile([128, NC], f32, tag="op0")
            Op1 = psO.tile([128, NC], f32, tag="op1")
            nc.vector.memset(Op0[:], 0.0)
            nc.vector.memset(Op1[:], 0.0)

            def emit_main(group, e):
                for t, lo, hi, off in group:
                    last = t == NT - 1
                    w = hi - lo
                    for bb, Opx in ((0, Op0), (1, Op1)):
                        nc.tensor.matmul(
                            Opx[:, lo:hi],
                            xfull[:, t * B + bb * 128 : t * B + (bb + 1) * 128],
                            e[:, off : off + w],
                            start=False,
                            stop=last,
                            skip_group_check=True,
                        )

            pending = []
            for group in groups:
                Gp = psG.tile([128, 512], f32, tag="Gp")
                for t, lo, hi, off in group:
                    nc.tensor.matmul(
                        Gp[:, off : off + (hi - lo)],
                        Wp[:, t * 128 : (t + 1) * 128],
                        Mv[:, lo:hi],
                        start=True,
                        stop=True,
                        skip_group_check=True,
                    )
                wg = group[-1][3] + group[-1][2] - group[-1][1]
                e = ep.tile([128, 512], bf16, tag="e")
                nc.scalar.activation(
                    e[:, 0:wg], Gp[:, 0:wg], mybir.ActivationFunctionType.Exp
                )
                pending.append((group, e))
                if len(pending) > skew:
                    emit_main(*pending.pop(0))
            while pending:
                emit_main(*pending.pop(0))

            out_sb = op.tile([128, 2 * NC], f32, tag="out")
            nc.scalar.copy(out_sb[:, 0:NC], Op0[:])
            nc.vector.tensor_copy(out_sb[:, NC : 2 * NC], Op1[:])
            nc.sync.dma_start(d_out[0], out_sb[:, 0:NC])
            nc.scalar.dma_start(d_out[1], out_sb[:, NC : 2 * NC])

    nc.compile()
    return nc


def _prepare(x, positions, sigmas, curve_weights, xs, ys):
    x = np.asarray(x, dtype=np.float32)
    px = np.asarray(positions, dtype=np.float64)[0, 0, :, 1]
    py = np.asarray(positions, dtype=np.float64)[0, 0, :, 0]
    sg = np.asarray(sigmas, dtype=np.float64)[0, 0]
    w = np.asarray(curve_weights, dtype=np.float64)[0, 0]
    xs = np.asarray(xs, dtype=np.float64)
    ys = np.asarray(ys, dtype=np.float64)

    s2 = 2.0 * sg * sg + EPS
    factor = 1.0 / (2.0 * np.pi * sg * sg + EPS)
    fw = factor * w
    # clip(curves) is identity when max|factor*w| <= CAP since exp(...) <= 1
    assert np.abs(fw).max() <= CAP, "clip binds; folded-scale scheme invalid"

    absfw = np.maximum(np.abs(fw), 1e-12)
    lnr = np.log(absfw / absfw.max())
    Tk = np.maximum(T_KEEP + lnr, T_FLOOR)
    Tw = np.maximum(T_WIN + lnr, T_FLOOR)

    gorder = np.argsort(px, kind="stable")
    grank = np.empty(C, dtype=np.int64)
    grank[gorder] = np.arange(C)

    los = np.full((NDEV, NT), np.iinfo(np.int64).max, dtype=np.int64)
    his = np.zeros((NDEV, NT), dtype=np.int64)
    band = []
    for d in range(NDEV):
        h0 = d * ROWS
        y0, y1 = ys[h0, 0], ys[h0 + ROWS - 1, 0]
        ymarg = np.maximum(np.maximum(y0 - py, py - y1), 0.0)
        kept = np.where(ymarg * ymarg / s2 <= Tk)[0]
        order = kept[np.argsort(px[kept], kind="stable")]
        nk = len(order)
        assert nk <= NC, f"band {d} keeps {nk} > NC={NC} columns"
        # monotone slot assignment following the global px-quantile so the
        # per-tile windows line up across bands
        ideal = (grank[order] * NC) // C
        slot = np.zeros(nk, dtype=np.int64)
        s = -1
        for i in range(nk):
            s = max(s + 1, int(ideal[i]))
            slot[i] = s
        if nk and slot[-1] > NC - 1:
            slot[-1] = NC - 1
            for i in range(nk - 2, -1, -1):
                slot[i] = min(slot[i], slot[i + 1] - 1)
        pxs = px[order]
        ym = ymarg[order]
        for t in range(NT):
            xi0, xi1 = (t * 128) // ROWS, (t * 128 + 127) // ROWS
            xx0, xx1 = xs[0, xi0], xs[0, min(xi1, W - 1)]
            xmarg = np.maximum(np.maximum(xx0 - pxs, pxs - xx1), 0.0)
            act = np.where((ym * ym + xmarg * xmarg) / s2[order] <= Tw[order])[0]
            if len(act):
                los[d, t] = slot[act[0]]
                his[d, t] = slot[act[-1]] + 1
        band.append((order, slot))

    lo_u = los.min(axis=0)
    hi_u = his.max(axis=0)
    # enforce monotone windows (they already are, up to ties) so every slot
    # in [lo_0, hi_last) is covered by a contiguous run of tiles
    for t in range(1, NT):
        hi_u[t] = max(hi_u[t], hi_u[t - 1]) if hi_u[t] else hi_u[t - 1]
        lo_u[t] = max(min(lo_u[t], hi_u[t]), lo_u[t - 1])
    windows = [
        (int(min(lo_u[t], hi_u[t])), int(hi_u[t])) for t in range(NT)
    ]

    in_maps = []
    gathers = []
    for d in range(NDEV):
        h0 = d * ROWS
        rows = slice(h0, h0 + ROWS)
        # x-major pixel order: p = xi*ROWS + yi
        xs_b = xs[rows].T.ravel()
        ys_b = ys[rows].T.ravel()
        Bm = np.stack(
            [xs_b, ys_b, np.ones(HWD), xs_b * xs_b + ys_b * ys_b]
        )

        order, slot = band[d]
        lnf = np.log(np.abs(fw[order]) + 1e-300) - np.log(NPIX)
        Am = np.zeros((4, NC))
        Am[2, :] = -60.0
        Am[3, :] = -1.0
        Am[0, slot] = 2.0 * px[order] / s2[order]
        Am[1, slot] = 2.0 * py[order] / s2[order]
        Am[2, slot] = -(px[order] ** 2 + py[order] ** 2) / s2[order] + lnf
        Am[3, slot] = -1.0 / s2[order]
        Bh = Bm.astype(np.float16)
        Bl = (Bm - Bh.astype(np.float64)).astype(np.float16)
        Ah = Am.astype(np.float16)
        Al = (Am - Ah.astype(np.float64)).astype(np.float16)
        # K=12 stacked hi/lo split: [Bh;Bh;Bl]^T @ [Ah;Al;Ah]
        Wp = np.concatenate([Bh, Bh, Bl], axis=0)
        Mv = np.concatenate([Ah, Al, Ah], axis=0)

        xT = np.ascontiguousarray(
            x[:, rows, :].transpose(0, 2, 1).reshape(B, HWD).T
        ).reshape(NT, 128, B).astype(ml_dtypes.bfloat16)

        in_maps.append({"xT": xT, "Wp": Wp, "Mv": Mv})
        gathers.append((order, slot, np.sign(fw[order]).astype(np.float32)))
    return windows, in_maps, gathers


def _gather(results, gathers):
    out = np.zeros((B, C), np.float32)
    for d in range(NDEV):
        order, slot, sgn = gathers[d]
        dev = np.asarray(results[d]["out"], np.float32).reshape(B, NC)
        out[:, order] += dev[:, slot] * sgn
    return out


def kernel(x, positions, sigmas, curve_weights, xs, ys):
    global last_results
    windows, in_maps, gathers = _prepare(
        x, positions, sigmas, curve_weights, xs, ys
    )
    nc = _build_program(windows)
    trace = bool(os.environ.get("BLOB_TRACE"))
    last_results = run_bass_kernel_spmd(
        nc, in_maps, list(range(NDEV)), trace=trace
    )
    return _gather(last_results.results, gathers)
